# Initial kernel scaffold
#
"""Hough-transform voting kernel for Trainium2 (8 NeuronCores).

out[m, b] = (1/128) * sum_i w_i * x[m, p_i] * [bin_i == b],  m in 0..31 maps,
b in 0..33119 bins (184x180), 4M votes.

Strategy:
  - Shard votes 8 ways across NeuronCores (all 32 maps on every core).
  - Host prep (index/weight layout only; never touches x values):
      counting-sort each shard's votes by bin; pack into "vbins" of 16 slots
      (a bin with c votes uses ceil(c/16) vbins); weights are placed in a
      one-hot W grid column (vbin % 128) so each 128-slot chunk maps into a
      128-vbin psum tile.
  - Device: dma_gather pulls 256B rows of the transposed x table (XT[pixel,
    128] bf16) so each slot's 32 map-values land on its own partition; PE
    matmuls contract slots: psum[128 vbins, 32 maps] accumulates
    W_chunk^T @ V_chunk over 16 chunks (PSUM start/stop); tiles are copied
    into an SBUF HT and DMA'd out once.
  - Host: add per-NC partials into the real bins (vbin -> bin map).
"""

import numpy as np

IM_H, IM_W = 128, 128
HT_H, HT_W = 184, 180
NB = HT_H * HT_W          # 33120 bins
NPIX = IM_H * IM_W        # 16384 pixels
NMAPS = 32
NCORES = 8
NORM = 128.0
SLOTS_PER_VBIN = 8
VBINS_PER_CHUNK = 16      # 128 slots per chunk
GROUP_SLOTS = 8192        # slots per dma_gather call (64 chunks, 4 psum tiles)


def _build_shard(vp, vb, vw):
    """Pack one shard's votes into the slot grid. Returns (pix_slots int16,
    w_slots f32, vbin2bin int32)."""
    order = np.argsort(vb, kind="stable")
    bins_s = vb[order]
    pix_s = vp[order]
    w_s = vw[order]
    counts = np.bincount(bins_s, minlength=NB)
    nvb = (counts + SLOTS_PER_VBIN - 1) // SLOTS_PER_VBIN
    vb_start = np.concatenate([[0], np.cumsum(nvb)])
    starts = np.concatenate([[0], np.cumsum(counts)])
    rank = np.arange(len(bins_s)) - starts[bins_s]
    slot = (vb_start[bins_s] + rank // SLOTS_PER_VBIN) * SLOTS_PER_VBIN + (
        rank % SLOTS_PER_VBIN
    )
    nvbins = int(vb_start[-1])
    vbin2bin = np.repeat(np.arange(NB), nvb).astype(np.int32)
    pix_slots = np.zeros(nvbins * SLOTS_PER_VBIN, np.int16)
    w_slots = np.zeros(nvbins * SLOTS_PER_VBIN, np.float32)
    pix_slots[slot] = pix_s.astype(np.int16)
    w_slots[slot] = w_s / NORM
    return pix_slots, w_slots, vbin2bin


def _pad_to(a, n, fill=0):
    out = np.full((n,) + a.shape[1:], fill, a.dtype)
    out[: len(a)] = a
    return out


def kernel(**inputs):
    import concourse.bacc as bacc
    import concourse.mybir as mybir
    import concourse.tile as tile
    from concourse import bass_utils

    bf16 = mybir.dt.np(mybir.dt.bfloat16)

    x = np.asarray(inputs["x"]).astype(np.float32)
    vp = np.asarray(inputs["vote_pixel"]).astype(np.int64)
    vb = np.asarray(inputs["vote_bin"]).astype(np.int64)
    vw = np.asarray(inputs["vote_weight"]).astype(np.float32)
    b, c = x.shape[0], x.shape[1]
    xf = x.reshape(b * c, NPIX)  # [32, 16384]

    # XT rows are the gather elements: 128 bf16 = 256 B. Cols 0-31 hold
    # bf16(x), cols 32-63 the bf16 residual (x - bf16(x)) so the matmul can
    # recover ~fp32 accuracy from the same 256B gather.
    xt = np.zeros((NPIX, 128), bf16)
    xhi = xf.T.astype(bf16)
    xt[:, :NMAPS] = xhi
    xt[:, NMAPS:2 * NMAPS] = (xf.T - xhi.astype(np.float32)).astype(bf16)

    # Static one-hot mask: slot s = 128c+p has vbin 16c+p//8; its psum column
    # is vbin % 128 = (16*cm + p//8) % 128 for cm = chunk-within-group.
    P, CM = np.meshgrid(np.arange(128), np.arange(64), indexing="ij")
    mask = np.zeros((128, 64, 128), np.float32)
    mask[P, CM, (16 * CM + P // 8) % 128] = 1.0
    mask = mask.reshape(128, 64 * 128).astype(bf16)

    shards = []
    for s in range(NCORES):
        shards.append(_build_shard(vp[s::NCORES], vb[s::NCORES], vw[s::NCORES]))
    nslot = max(len(p) for p, _, _ in shards)
    nslot = ((nslot + GROUP_SLOTS - 1) // GROUP_SLOTS) * GROUP_SLOTS
    ngroups = nslot // GROUP_SLOTS
    nchunk = nslot // 128
    ntile = nchunk // 8  # 128-vbin psum tiles

    in_maps = []
    vb2b = []
    for pix_slots, w_slots, vbin2bin in shards:
        pix_slots = _pad_to(pix_slots, nslot)
        w_slots = _pad_to(w_slots, nslot)
        vb2b.append(vbin2bin)
        # dma_gather on HW reads the idx stream from partitions 16..31
        # (Q7 core 1); the simulator reads 0..15. Feed both the same data.
        idxw = pix_slots.reshape(-1, 16).T  # [16, nslot/16], vote j = [j%16, j//16]
        idx_tile = np.concatenate([idxw, idxw], axis=0).astype(np.int16)
        wc = np.ascontiguousarray(w_slots.reshape(nchunk, 128).T).astype(bf16)
        in_maps.append({"xt": xt, "idx": idx_tile, "wc": wc, "mask": mask})

    # ---- build the (single, SPMD) device program ----
    global _PROG_CACHE
    try:
        cached = _PROG_CACHE
    except NameError:
        cached = _PROG_CACHE = {}
    globals()["_LAST_IN_MAPS"] = in_maps
    if nslot in cached:
        nc = cached[nslot]
        res = bass_utils.run_bass_kernel_spmd(nc, in_maps, core_ids=list(range(NCORES)))
        return _combine(res, vb2b, ntile, b, c)
    nc = bacc.Bacc("TRN2", target_bir_lowering=False, debug=False)
    xt_d = nc.dram_tensor("xt", [NPIX, 128], mybir.dt.bfloat16, kind="ExternalInput")
    idx_d = nc.dram_tensor("idx", [32, nslot // 16], mybir.dt.int16, kind="ExternalInput")
    wc_d = nc.dram_tensor("wc", [128, nchunk], mybir.dt.bfloat16, kind="ExternalInput")
    mask_d = nc.dram_tensor(
        "mask", [128, 64 * 128], mybir.dt.bfloat16, kind="ExternalInput"
    )
    ht_d = nc.dram_tensor(
        "ht", [128, ntile * NMAPS], mybir.dt.float32, kind="ExternalOutput"
    )

    gcols = GROUP_SLOTS // 16              # idx cols per group
    wcols = (GROUP_SLOTS // 128) * 128     # W cols per group

    with tile.TileContext(nc) as tc:
        with (
            tc.tile_pool(name="idxp", bufs=2) as idxp,
            tc.tile_pool(name="vp", bufs=2) as vpool,
            tc.tile_pool(name="wp", bufs=2) as wpool,
            tc.tile_pool(name="htp", bufs=1) as htp,
            tc.tile_pool(name="ps", bufs=4, space="PSUM") as psp,
        ):
            ht_sb = htp.tile([128, ntile * NMAPS], mybir.dt.float32)
            mask_sb = htp.tile([128, 64, 128], mybir.dt.bfloat16)
            nc.sync.dma_start(mask_sb[:], mask_d[:])
            for g in range(ngroups):
                idx_sb = idxp.tile([32, gcols], mybir.dt.int16, tag="i")
                nc.sync.dma_start(idx_sb[:], idx_d[:, g * gcols:(g + 1) * gcols])
                v_sb = vpool.tile([128, GROUP_SLOTS // 128, 128], mybir.dt.bfloat16,
                                  tag="v")
                nc.gpsimd.dma_gather(
                    v_sb[:], xt_d[:], idx_sb[:],
                    GROUP_SLOTS, GROUP_SLOTS, 128, single_packet=False,
                )
                wc_sb = wpool.tile([128, 64], mybir.dt.bfloat16, tag="wc")
                nc.sync.dma_start(wc_sb[:], wc_d[:, g * 64:(g + 1) * 64])
                w_sb = wpool.tile([128, 64, 128], mybir.dt.bfloat16, tag="w")
                nc.vector.tensor_tensor(
                    out=w_sb[:],
                    in0=wc_sb[:].to_broadcast([128, 64, 128]),
                    in1=mask_sb[:],
                    op=mybir.AluOpType.mult,
                )
                for pt in range(8):  # psum tiles in this group
                    psum = psp.tile([128, NMAPS], mybir.dt.float32, space="PSUM")
                    for h in range(2):  # x-hi then x-lo residual columns
                        for k in range(8):
                            row = pt * 8 + k
                            nc.tensor.matmul(
                                psum[:, :],
                                lhsT=w_sb[:, row, :],
                                rhs=v_sb[:, row, h * NMAPS:(h + 1) * NMAPS],
                                start=(h == 0 and k == 0), stop=(h == 1 and k == 7),
                            )
                    t = g * 8 + pt
                    nc.vector.tensor_copy(
                        ht_sb[:, t * NMAPS:(t + 1) * NMAPS], psum[:]
                    )
            nc.sync.dma_start(ht_d[:], ht_sb[:])
    nc.compile()
    cached[nslot] = nc

    res = bass_utils.run_bass_kernel_spmd(nc, in_maps, core_ids=list(range(NCORES)))
    return _combine(res, vb2b, ntile, b, c)


def _combine(res, vb2b, ntile, b, c):
    out = np.zeros((NB + 1, NMAPS), np.float32)
    for s in range(NCORES):
        ht = res.results[s]["ht"]  # [128, ntile*32]
        partial = ht.reshape(128, ntile, NMAPS).transpose(1, 0, 2).reshape(-1, NMAPS)
        nvb = len(vb2b[s])
        np.add.at(out, vb2b[s], partial[:nvb])
    return np.ascontiguousarray(out[:NB].T).reshape(b, c, HT_H, HT_W)



# revision 1
# speedup vs baseline: 5.6230x; 5.6230x over previous
"""Hough-transform voting kernel for Trainium2 (8 NeuronCores).

out[m, b] = (1/128) * sum_i w_i * x[m, p_i] * [bin_i == b],  m in 0..31 maps,
b in 0..33119 bins (184x180), 4M votes.

Strategy:
  - Shard votes 8 ways across NeuronCores (all 32 maps on every core).
  - Host prep (index/weight layout only; never touches x values):
      counting-sort each shard's votes by bin; pack into "vbins" of 16 slots
      (a bin with c votes uses ceil(c/16) vbins); weights are placed in a
      one-hot W grid column (vbin % 128) so each 128-slot chunk maps into a
      128-vbin psum tile.
  - Device: dma_gather pulls 256B rows of the transposed x table (XT[pixel,
    128] bf16) so each slot's 32 map-values land on its own partition; PE
    matmuls contract slots: psum[128 vbins, 32 maps] accumulates
    W_chunk^T @ V_chunk over 16 chunks (PSUM start/stop); tiles are copied
    into an SBUF HT and DMA'd out once.
  - Host: add per-NC partials into the real bins (vbin -> bin map).
"""

import numpy as np

IM_H, IM_W = 128, 128
HT_H, HT_W = 184, 180
NB = HT_H * HT_W          # 33120 bins
NPIX = IM_H * IM_W        # 16384 pixels
NMAPS = 32
NCORES = 8
NORM = 128.0
SLOTS_PER_VBIN = 8
VBINS_PER_CHUNK = 16      # 128 slots per chunk
GROUP_SLOTS = 8192        # slots per dma_gather call (64 chunks, 4 psum tiles)


def _build_shard(vp, vb, vw):
    """Pack one shard's votes into the slot grid. Returns (pix_slots int16,
    w_slots f32, vbin2bin int32)."""
    order = np.argsort(vb, kind="stable")
    bins_s = vb[order]
    pix_s = vp[order]
    w_s = vw[order]
    counts = np.bincount(bins_s, minlength=NB)
    nvb = (counts + SLOTS_PER_VBIN - 1) // SLOTS_PER_VBIN
    vb_start = np.concatenate([[0], np.cumsum(nvb)])
    starts = np.concatenate([[0], np.cumsum(counts)])
    rank = np.arange(len(bins_s)) - starts[bins_s]
    slot = (vb_start[bins_s] + rank // SLOTS_PER_VBIN) * SLOTS_PER_VBIN + (
        rank % SLOTS_PER_VBIN
    )
    nvbins = int(vb_start[-1])
    vbin2bin = np.repeat(np.arange(NB), nvb).astype(np.int32)
    pix_slots = np.zeros(nvbins * SLOTS_PER_VBIN, np.int16)
    w_slots = np.zeros(nvbins * SLOTS_PER_VBIN, np.float32)
    pix_slots[slot] = pix_s.astype(np.int16)
    w_slots[slot] = w_s / NORM
    return pix_slots, w_slots, vbin2bin


def _pad_to(a, n, fill=0):
    out = np.full((n,) + a.shape[1:], fill, a.dtype)
    out[: len(a)] = a
    return out


def kernel(**inputs):
    import concourse.bacc as bacc
    import concourse.mybir as mybir
    import concourse.tile as tile
    from concourse import bass_utils

    bf16 = mybir.dt.np(mybir.dt.bfloat16)

    x = np.asarray(inputs["x"]).astype(np.float32)
    vp = np.asarray(inputs["vote_pixel"]).astype(np.int64)
    vb = np.asarray(inputs["vote_bin"]).astype(np.int64)
    vw = np.asarray(inputs["vote_weight"]).astype(np.float32)
    b, c = x.shape[0], x.shape[1]
    xf = x.reshape(b * c, NPIX)  # [32, 16384]

    # XT rows are the gather elements: 128 bf16 = 256 B. Cols 0-31 hold
    # bf16(x), cols 32-63 the bf16 residual (x - bf16(x)) so the matmul can
    # recover ~fp32 accuracy from the same 256B gather.
    xt = np.zeros((NPIX, 128), bf16)
    xhi = xf.T.astype(bf16)
    xt[:, :NMAPS] = xhi
    xt[:, NMAPS:2 * NMAPS] = (xf.T - xhi.astype(np.float32)).astype(bf16)

    # Static one-hot mask: slot s = 128c+p has vbin 16c+p//8; its psum column
    # is vbin % 128 = (16*cm + p//8) % 128 for cm = chunk-within-group.
    P, CM = np.meshgrid(np.arange(128), np.arange(64), indexing="ij")
    mask = np.zeros((128, 64, 128), np.float32)
    mask[P, CM, (16 * CM + P // 8) % 128] = 1.0
    mask = mask.reshape(128, 64 * 128).astype(bf16)

    shards = []
    for s in range(NCORES):
        shards.append(_build_shard(vp[s::NCORES], vb[s::NCORES], vw[s::NCORES]))
    nslot = max(len(p) for p, _, _ in shards)
    nslot = ((nslot + GROUP_SLOTS - 1) // GROUP_SLOTS) * GROUP_SLOTS
    ngroups = nslot // GROUP_SLOTS
    nchunk = nslot // 128
    ntile = nchunk // 8  # 128-vbin psum tiles

    in_maps = []
    vb2b = []
    for pix_slots, w_slots, vbin2bin in shards:
        pix_slots = _pad_to(pix_slots, nslot)
        w_slots = _pad_to(w_slots, nslot)
        vb2b.append(vbin2bin)
        # dma_gather on HW reads the idx stream from partitions 16..31
        # (Q7 core 1); the simulator reads 0..15. Feed both the same data.
        idxw = pix_slots.reshape(-1, 16).T  # [16, nslot/16], vote j = [j%16, j//16]
        idx_tile = np.concatenate([idxw, idxw], axis=0).astype(np.int16)
        wc = np.ascontiguousarray(w_slots.reshape(nchunk, 128).T).astype(bf16)
        in_maps.append({"xt": xt, "idx": idx_tile, "wc": wc, "mask": mask})

    # ---- build the (single, SPMD) device program ----
    global _PROG_CACHE
    try:
        cached = _PROG_CACHE
    except NameError:
        cached = _PROG_CACHE = {}
    globals()["_LAST_IN_MAPS"] = in_maps
    if nslot in cached:
        nc = cached[nslot]
        res = bass_utils.run_bass_kernel_spmd(nc, in_maps, core_ids=list(range(NCORES)))
        return _combine(res, vb2b, ntile, b, c)
    nc = bacc.Bacc("TRN2", target_bir_lowering=False, debug=False)
    xt_d = nc.dram_tensor("xt", [NPIX, 128], mybir.dt.bfloat16, kind="ExternalInput")
    idx_d = nc.dram_tensor("idx", [32, nslot // 16], mybir.dt.int16, kind="ExternalInput")
    wc_d = nc.dram_tensor("wc", [128, nchunk], mybir.dt.bfloat16, kind="ExternalInput")
    mask_d = nc.dram_tensor(
        "mask", [128, 64 * 128], mybir.dt.bfloat16, kind="ExternalInput"
    )
    ht_d = nc.dram_tensor(
        "ht", [128, ntile * NMAPS], mybir.dt.float32, kind="ExternalOutput"
    )

    gcols = GROUP_SLOTS // 16              # idx cols per group
    wcols = (GROUP_SLOTS // 128) * 128     # W cols per group

    with tile.TileContext(nc) as tc:
        with (
            tc.tile_pool(name="idxp", bufs=2) as idxp,
            tc.tile_pool(name="vp", bufs=2) as vpool,
            tc.tile_pool(name="wp", bufs=2) as wpool,
            tc.tile_pool(name="htp", bufs=1) as htp,
            tc.tile_pool(name="ps", bufs=4, space="PSUM") as psp,
        ):
            ht_sb = htp.tile([128, ntile * NMAPS], mybir.dt.float32)
            mask_sb = htp.tile([128, 64, 128], mybir.dt.bfloat16)
            nc.sync.dma_start(mask_sb[:], mask_d[:])
            for g in range(ngroups):
                idx_sb = idxp.tile([32, gcols], mybir.dt.int16, tag="i")
                nc.sync.dma_start(idx_sb[:], idx_d[:, g * gcols:(g + 1) * gcols])
                v_sb = vpool.tile([128, GROUP_SLOTS // 128, 128], mybir.dt.bfloat16,
                                  tag="v")
                nc.gpsimd.dma_gather(
                    v_sb[:], xt_d[:], idx_sb[:],
                    GROUP_SLOTS, GROUP_SLOTS, 128, single_packet=False,
                )
                wc_sb = wpool.tile([128, 64], mybir.dt.bfloat16, tag="wc")
                nc.sync.dma_start(wc_sb[:], wc_d[:, g * 64:(g + 1) * 64])
                w_sb = wpool.tile([128, 64, 128], mybir.dt.bfloat16, tag="w")
                nc.vector.tensor_tensor(
                    out=w_sb[:],
                    in0=wc_sb[:].to_broadcast([128, 64, 128]),
                    in1=mask_sb[:],
                    op=mybir.AluOpType.mult,
                )
                for pt in range(8):  # psum tiles in this group
                    psum = psp.tile([128, NMAPS], mybir.dt.float32, space="PSUM")
                    for h in range(2):  # x-hi then x-lo residual columns
                        for k in range(8):
                            row = pt * 8 + k
                            nc.tensor.matmul(
                                psum[:, :],
                                lhsT=w_sb[:, row, :],
                                rhs=v_sb[:, row, h * NMAPS:(h + 1) * NMAPS],
                                start=(h == 0 and k == 0), stop=(h == 1 and k == 7),
                            )
                    t = g * 8 + pt
                    nc.vector.tensor_copy(
                        ht_sb[:, t * NMAPS:(t + 1) * NMAPS], psum[:]
                    )
            nc.sync.dma_start(ht_d[:], ht_sb[:])
    nc.compile()
    cached[nslot] = nc

    res = bass_utils.run_bass_kernel_spmd(nc, in_maps, core_ids=list(range(NCORES)))
    return _combine(res, vb2b, ntile, b, c)


def _combine(res, vb2b, ntile, b, c):
    out = np.zeros((NB + 1, NMAPS), np.float32)
    for s in range(NCORES):
        ht = res.results[s]["ht"]  # [128, ntile*32]
        partial = ht.reshape(128, ntile, NMAPS).transpose(1, 0, 2).reshape(-1, NMAPS)
        nvb = len(vb2b[s])
        np.add.at(out, vb2b[s], partial[:nvb])
    return np.ascontiguousarray(out[:NB].T).reshape(b, c, HT_H, HT_W)

